# revision 1
# baseline (speedup 1.0000x reference)
"""MinGRU block kernel for Trainium2 (Bass/Tile), SPMD over 8 NeuronCores.

Problem: B=8, S=2048, D=1024, F=3072 (nn_MinGRUBlock).
Sharding: data-parallel over batch (one batch row per core); weights replicated.

Per-core dataflow (all compute in "T layout": feature on partitions, time on free):
  phase 1 (mixer, s-chunks of 256):
    load x chunk [s,d] -> PE-transpose -> xT [d,s]
    rmsnorm row-scales r computed via squares + PE ones-reduce + sqrt/recip
    r broadcast across partitions via K=1 PE matmul
    g/v/d projections: fp16 matmuls (1 cyc/row on PE), fp32 PSUM accumulate
    sigmoid/tanh on ACT directly from PSUM (bias fused)
    h_t = a_t*h_{t-1} + x_t via DVE tensor_tensor_scan (fp32 state), chained
    across chunks with a carry column
    out1 = x + h; out1 and normalized out1n bounced to DRAM scratch
  phase 2 (FFN): stream W_gate/W_up once, z = silu(gate)*up in fp16 (12MB SBUF),
    then W_out matmuls + residual, PE-transpose back to [s,d], DMA out.
"""

import os
import sys
from contextlib import ExitStack

import numpy as np

for _p in ("/opt/trn_rl_repo", "/root/.axon_site/_ro/trn_rl_repo"):
    if os.path.isdir(_p) and _p not in sys.path:
        sys.path.insert(0, _p)

import concourse.bass as bass
import concourse.tile as tile
from concourse import bacc, mybir
from concourse.bass_utils import run_bass_kernel_spmd

F32 = mybir.dt.float32
F32R = mybir.dt.float32r
F16 = mybir.dt.float16
AF = mybir.ActivationFunctionType
OP = mybir.AluOpType

B, S, D, F = 8, 2048, 1024, 3072
EPS = 1e-6
KD = D // 128          # 8 d-ptiles
MF2 = 2 * F // 128     # 48 f-ptiles (gate|up)
MFO = F // 128         # 24 f-ptiles
MD = D // 128          # 8 d-ptiles (output)

CH1 = 256              # phase-1 s-chunk
NCH1 = S // CH1
CH2 = 512              # phase-2 s-chunk
NCH2 = S // CH2
NST1 = CH1 // 128      # s-tiles per phase-1 chunk


def build_program():
    nc = bacc.Bacc("TRN2", target_bir_lowering=False, debug=False)

    x_d = nc.dram_tensor("x", [S, D], F32, kind="ExternalInput").ap()
    wmix_d = nc.dram_tensor("w_mix", [3 * MD, 128, KD, 128], F16, kind="ExternalInput").ap()
    bmix_d = nc.dram_tensor("b_mix", [128, 3 * MD], F32, kind="ExternalInput").ap()
    wgu_d = nc.dram_tensor("w_gu", [MF2, 128, KD, 128], F16, kind="ExternalInput").ap()
    wout_d = nc.dram_tensor("w_out", [MD, 128, MFO, 128], F16, kind="ExternalInput").ap()
    ident_d = nc.dram_tensor("ident", [128, 128], F32, kind="ExternalInput").ap()
    out_d = nc.dram_tensor("out", [S, D], F32, kind="ExternalOutput").ap()

    with tile.TileContext(nc) as tc, ExitStack() as top:
        # ---------- persistent tiles ----------
        cpool = top.enter_context(tc.tile_pool(name="consts", bufs=1))
        ident = cpool.tile([128, 128], F32)
        nc.sync.dma_start(ident[:], ident_d[:])
        ones_col = cpool.tile([128, 1], F16)
        nc.vector.memset(ones_col[:], 1.0)
        ones_row = cpool.tile([1, 128], F32)
        nc.vector.memset(ones_row[:], 1.0)
        ones_row_r = cpool.tile([1, 128], F32R)
        nc.vector.tensor_copy(ones_row_r[:], ones_row[:])
        zero128 = cpool.tile([128, 1], F32)
        nc.vector.memset(zero128[:], 0.0)
        eps1 = cpool.tile([1, 1], F32)
        nc.vector.memset(eps1[:], EPS)
        bmix = cpool.tile([128, 3 * MD], F32)
        nc.sync.dma_start(bmix[:], bmix_d[:])

        # DRAM scratch (tile-tracked so phase-2 reads order after phase-1
        # writes). One tile per phase-2 s-chunk so a phase-2 load only
        # depends on the phase-1 chunks that actually wrote it.
        dpool = top.enter_context(tc.tile_pool(name="dscratch", bufs=1, space="DRAM"))
        sc1_t = [dpool.tile([KD, 128, CH2], F32, name=f"sc1_{i}") for i in range(NCH2)]
        sc1_p = [t.rearrange("k p s -> p k s") for t in sc1_t]

        # normalized out1 stays resident in SBUF across phase 1 -> 2a
        o1n_pool = top.enter_context(tc.tile_pool(name="o1n", bufs=1))
        o1n = o1n_pool.tile([128, KD, S], F16)

        carry_pool = top.enter_context(tc.tile_pool(name="carry", bufs=1))
        carry = carry_pool.tile([128, KD], F32)

        # ---------- phase 1: mixer (software-pipelined over chunks) ----------
        with ExitStack() as ph1:
            wpool = ph1.enter_context(tc.tile_pool(name="wmix", bufs=1))
            wmix = wpool.tile([128, 3 * MD, KD, 128], F16)
            wmix_dp = wmix_d.rearrange("m p k j -> p m k j")

            p_nat = ph1.enter_context(tc.tile_pool(name="xnat", bufs=3))
            p_xT = ph1.enter_context(tc.tile_pool(name="xT", bufs=3))
            p_16 = ph1.enter_context(tc.tile_pool(name="f16bufs", bufs=2))
            p_sq = ph1.enter_context(tc.tile_pool(name="sqbufs", bufs=3))
            p_32 = ph1.enter_context(tc.tile_pool(name="f32bufs", bufs=2))
            p_row = ph1.enter_context(tc.tile_pool(name="rows", bufs=2))
            ps_tp = ph1.enter_context(tc.tile_pool(name="tp_ps", bufs=2, space="PSUM"))
            ps_mm = ph1.enter_context(tc.tile_pool(name="mm_ps", bufs=3, space="PSUM"))
            ps_ss = ph1.enter_context(tc.tile_pool(name="ss_ps", bufs=1, space="PSUM"))
            ps_bc = ph1.enter_context(tc.tile_pool(name="bc_ps", bufs=2, space="PSUM"))

            st_front = {}   # c -> (xT, rrow1)
            st_bc1 = {}     # c -> bc1 psum tile
            st_body = {}    # c -> (sig_g, tanh_v, sig_d)
            st_back = {}    # c -> (out1, rrow2)

            st_sq1 = {}

            def front_t(c):
                """load + transpose x chunk, squares (ACT)."""
                s0 = c * CH1
                xT = p_xT.tile([128, KD, CH1], F32, tag="xT", name=f"xT{c}")
                sq = p_sq.tile([128, KD, CH1], F16, tag="sq", name=f"sq1_{c}")
                nats = []
                for st in range(NST1):
                    xn_t = p_nat.tile([128, D], F32, tag="xnat", name=f"xnat{c}_{st}")
                    nc.sync.dma_start(xn_t[:], x_d[s0 + st * 128: s0 + (st + 1) * 128, :])
                    nats.append(xn_t)
                for kt in range(KD):
                    tp = ps_tp.tile([128, CH1], F32, tag="tp", name=f"tp{c}_{kt}")
                    for st in range(NST1):
                        nc.tensor.transpose(tp[:, st * 128:(st + 1) * 128],
                                            nats[st][:, kt * 128:(kt + 1) * 128],
                                            ident[:])
                    nc.scalar.copy(xT[:, kt], tp[:])
                    nc.scalar.activation(sq[:, kt], xT[:, kt], AF.Square,
                                         bias=zero128[:])
                st_front[c] = (xT, None)
                st_sq1[c] = sq

            def front_r(c):
                """norm1 reduce (PE) + sqrt/recip."""
                sq = st_sq1[c]
                ss = ps_ss.tile([1, CH1], F32, tag="ss", name=f"ss1_{c}")
                for kt in range(KD):
                    nc.tensor.matmul(ss[:], ones_col[:], sq[:, kt],
                                     start=(kt == 0), stop=(kt == KD - 1))
                srow = p_row.tile([1, CH1], F32, tag="srow1", name=f"srow1_{c}")
                nc.scalar.activation(srow[:], ss[:], AF.Sqrt, bias=eps1[:], scale=1.0 / D)
                rrow = p_row.tile([1, CH1], F32, tag="rrow1", name=f"rrow1_{c}")
                nc.vector.reciprocal(rrow[:], srow[:])
                st_front[c] = (st_front[c][0], rrow)

            def bcast1(c):
                rrow = st_front[c][1]
                bc = ps_bc.tile([128, CH1], F32, tag="bc", name=f"bc1_{c}")
                nc.tensor.matmul(bc[:], ones_row[:], rrow[:])
                st_bc1[c] = bc

            def body(c):
                """normalized input + g/v/d projections + activations."""
                xT = st_front[c][0]
                bc1 = st_bc1[c]
                xnT = p_16.tile([128, KD, CH1], F16, tag="xnT", name=f"xnT{c}")
                for kt in range(KD):
                    nc.vector.tensor_tensor(xnT[:, kt], xT[:, kt], bc1[:], OP.mult)
                sig_g = p_16.tile([128, KD, CH1], F16, tag="sig_g", name=f"sg{c}")
                tanh_v = p_16.tile([128, KD, CH1], F16, tag="tanh_v", name=f"tv{c}")
                sig_d = p_16.tile([128, KD, CH1], F16, tag="sig_d", name=f"sd{c}")
                for proj, (dst, fn) in enumerate(
                        ((sig_g, AF.Sigmoid), (tanh_v, AF.Tanh), (sig_d, AF.Sigmoid))):
                    for half in range(4):
                        ps = ps_mm.tile([128, 2, CH1], F32, tag="mm",
                                        name=f"mm{c}_{proj}_{half}")
                        for mi in range(2):
                            mt = proj * MD + half * 2 + mi
                            for kt in range(KD):
                                nc.tensor.matmul(ps[:, mi], wmix[:, mt, kt], xnT[:, kt],
                                                 start=(kt == 0), stop=(kt == KD - 1))
                        for mi in range(2):
                            mt = proj * MD + half * 2 + mi
                            nc.scalar.activation(dst[:, half * 2 + mi], ps[:, mi], fn,
                                                 bias=bmix[:, mt:mt + 1])
                st_body[c] = (sig_g, tanh_v, sig_d)

            def back_a(c):
                """scan inputs, scan, residual, norm2 squares+reduce.

                Per-kt pipeline so the norm2 PE reduce starts while later
                kt rows are still scanning on DVE."""
                sig_g, tanh_v, sig_d = st_body[c]
                xT = st_front[c][0]
                xs = p_16.tile([128, KD, CH1], F16, tag="xs", bufs=1, name=f"xs{c}")
                nc.vector.tensor_tensor(xs[:], sig_g[:], tanh_v[:], OP.mult)
                a_t = p_16.tile([128, KD, CH1], F16, tag="a_t", bufs=1, name=f"a{c}")
                nc.vector.tensor_scalar(a_t[:], sig_d[:], 0.998, 0.001, OP.mult, OP.add)
                hT = p_32.tile([128, KD, CH1], F32, tag="hT", bufs=1, name=f"hT{c}")
                out1 = p_32.tile([128, KD, CH1], F32, tag="out1", name=f"o1_{c}")
                sq = p_sq.tile([128, KD, CH1], F16, tag="sq", name=f"sq2_{c}")
                ss = ps_ss.tile([1, CH1], F32, tag="ss", name=f"ss2_{c}")
                for kt in range(KD):
                    init = 0.0 if c == 0 else carry[:, kt:kt + 1]
                    nc.vector.tensor_tensor_scan(hT[:, kt], a_t[:, kt], xs[:, kt],
                                                 init, OP.mult, OP.add)
                    nc.vector.tensor_copy(carry[:, kt:kt + 1], hT[:, kt, CH1 - 1:CH1])
                    nc.vector.tensor_tensor(out1[:, kt], xT[:, kt], hT[:, kt], OP.add)
                    nc.scalar.activation(sq[:, kt], out1[:, kt], AF.Square,
                                         bias=zero128[:])
                    nc.tensor.matmul(ss[:], ones_col[:], sq[:, kt],
                                     start=(kt == 0), stop=(kt == KD - 1))
                srow = p_row.tile([1, CH1], F32, tag="srow2", name=f"srow2_{c}")
                nc.scalar.activation(srow[:], ss[:], AF.Sqrt, bias=eps1[:], scale=1.0 / D)
                rrow = p_row.tile([1, CH1], F32, tag="rrow2", name=f"rrow2_{c}")
                nc.vector.reciprocal(rrow[:], srow[:])
                st_back[c] = (out1, rrow)

            def back_b(c):
                """norm2 broadcast, normalized out1 into resident SBUF, out1 bounce."""
                s0 = c * CH1
                out1, rrow = st_back[c]
                bc = ps_bc.tile([128, CH1], F32, tag="bc", name=f"bc2_{c}")
                nc.tensor.matmul(bc[:], ones_row[:], rrow[:])
                for kt in range(KD):
                    nc.vector.tensor_tensor(o1n[:, kt, s0:s0 + CH1], out1[:, kt],
                                            bc[:], OP.mult)
                sc, off = divmod(s0, CH2)
                nc.sync.dma_start(sc1_p[sc][:, :, off:off + CH1], out1[:])

            # pipelined emission; PE stream per cycle:
            #   [T(c+1)] [MM(c)] [R1(c+1)] [R2(c-1)] [B1(c+1)] [B2(c-1)]
            front_t(0)
            # mixer weights per e-ptile so the first projections start early
            for mt in range(3 * MD):
                nc.sync.dma_start(wmix[:, mt], wmix_dp[:, mt])
            front_r(0)
            bcast1(0)
            for c in range(NCH1):
                if c + 1 < NCH1:
                    front_t(c + 1)
                body(c)
                if c + 1 < NCH1:
                    front_r(c + 1)
                if c >= 1:
                    back_a(c - 1)
                if c + 1 < NCH1:
                    bcast1(c + 1)
                if c >= 1:
                    back_b(c - 1)
            back_a(NCH1 - 1)
            back_b(NCH1 - 1)

        # ---------- phase 2: FFN ----------
        with ExitStack() as ph2:
            zpool = ph2.enter_context(tc.tile_pool(name="zbuf", bufs=1))
            z = zpool.tile([128, MFO, S], F16)

            # 2a: gate/up + z
            with ExitStack() as ph2a:
                p_wgu = ph2a.enter_context(tc.tile_pool(name="wgu", bufs=4))
                p_gu = ph2a.enter_context(tc.tile_pool(name="gu16", bufs=3))
                ps_gu = ph2a.enter_context(tc.tile_pool(name="gu_ps", bufs=4, space="PSUM"))
                for mg in range(MFO):
                    wg = p_wgu.tile([128, KD, 128], F16, tag="wgu")
                    nc.sync.dma_start(wg[:], wgu_d[mg])
                    wu = p_wgu.tile([128, KD, 128], F16, tag="wgu")
                    nc.sync.dma_start(wu[:], wgu_d[MFO + mg])
                    for sc in range(NCH2):
                        sl = slice(sc * CH2, (sc + 1) * CH2)
                        gps = ps_gu.tile([128, CH2], F32, tag="gups")
                        for kt in range(KD):
                            nc.tensor.matmul(gps[:], wg[:, kt], o1n[:, kt, sl],
                                             start=(kt == 0), stop=(kt == KD - 1))
                        ups = ps_gu.tile([128, CH2], F32, tag="gups")
                        for kt in range(KD):
                            nc.tensor.matmul(ups[:], wu[:, kt], o1n[:, kt, sl],
                                             start=(kt == 0), stop=(kt == KD - 1))
                        sig = p_gu.tile([128, CH2], F16, tag="sig")
                        nc.scalar.activation(sig[:], gps[:], AF.Sigmoid, bias=zero128[:])
                        gate = p_gu.tile([128, CH2], F16, tag="gate")
                        nc.vector.tensor_tensor(gate[:], gps[:], sig[:], OP.mult)
                        up = p_gu.tile([128, CH2], F16, tag="up")
                        nc.scalar.copy(up[:], ups[:])
                        nc.vector.tensor_tensor(z[:, mg, sl], gate[:], up[:], OP.mult)

            # 2b: W_out + residual + transpose out
            with ExitStack() as ph2b:
                p_wo = ph2b.enter_context(tc.tile_pool(name="wout", bufs=3))
                p_o1c = ph2b.enter_context(tc.tile_pool(name="o1c", bufs=3))
                p_oT = ph2b.enter_context(tc.tile_pool(name="outT", bufs=MD + 1))
                p_onat = ph2b.enter_context(tc.tile_pool(name="onat", bufs=3))
                ps_y = ph2b.enter_context(tc.tile_pool(name="y_ps", bufs=2, space="PSUM"))
                ps_t2 = ph2b.enter_context(tc.tile_pool(name="t2_ps", bufs=2, space="PSUM"))
                for sc in range(NCH2):
                    sl = slice(sc * CH2, (sc + 1) * CH2)
                    outTs = []
                    for mo in range(MD):
                        wo = p_wo.tile([128, MFO, 128], F16, tag="wo",
                                       name=f"wo{sc}_{mo}")
                        nc.sync.dma_start(wo[:], wout_d[mo])
                        yps = ps_y.tile([128, CH2], F32, tag="yps")
                        for kt in range(MFO):
                            nc.tensor.matmul(yps[:], wo[:, kt], z[:, kt, sl],
                                             start=(kt == 0), stop=(kt == MFO - 1))
                        o1c = p_o1c.tile([128, CH2], F32, tag="o1c")
                        nc.sync.dma_start(o1c[:], sc1_t[sc][mo])
                        oT = p_oT.tile([128, CH2], F32, tag="oT")
                        nc.vector.tensor_tensor(oT[:], yps[:], o1c[:], OP.add)
                        outTs.append(oT)
                    for q in range(CH2 // 128):
                        onat = p_onat.tile([128, D], F32, tag="onat")
                        for h in range(2):
                            t2 = ps_t2.tile([128, 512], F32, tag="t2")
                            for j in range(4):
                                nc.tensor.transpose(
                                    t2[:, j * 128:(j + 1) * 128],
                                    outTs[4 * h + j][:, q * 128:(q + 1) * 128],
                                    ident[:])
                            nc.scalar.copy(onat[:, h * 512:(h + 1) * 512], t2[:])
                        srow0 = sc * CH2 + q * 128
                        nc.sync.dma_start(out_d[srow0:srow0 + 128, :], onat[:])

    nc.compile()
    return nc


_NC = None


def _get_nc():
    global _NC
    if _NC is None:
        _NC = build_program()
    return _NC


def _prep_weights(inputs):
    w1 = np.asarray(inputs["rms_mix_w"], np.float32)
    w2 = np.asarray(inputs["rms_ffn_w"], np.float32)
    Wg = np.asarray(inputs["Wg"], np.float32) * w1[None, :]
    Wv = np.asarray(inputs["Wv"], np.float32) * w1[None, :]
    Wd = np.asarray(inputs["Wd"], np.float32) * w1[None, :]
    Wcat = np.concatenate([Wg, Wv, Wd], axis=0)            # [3D, D]
    w_mix = np.ascontiguousarray(
        Wcat.T.reshape(KD, 128, 3 * MD, 128).transpose(2, 1, 0, 3)).astype(np.float16)
    bcat = np.concatenate([np.asarray(inputs["bg"], np.float32),
                           np.asarray(inputs["bv"], np.float32),
                           np.asarray(inputs["bd"], np.float32)])
    b_mix = np.ascontiguousarray(bcat.reshape(3 * MD, 128).T).astype(np.float32)
    Wgate = np.asarray(inputs["W_gate"], np.float32) * w2[None, :]
    Wup = np.asarray(inputs["W_up"], np.float32) * w2[None, :]
    Wcat2 = np.concatenate([Wgate, Wup], axis=0)           # [2F, D]
    w_gu = np.ascontiguousarray(
        Wcat2.T.reshape(KD, 128, MF2, 128).transpose(2, 1, 0, 3)).astype(np.float16)
    WoT = np.asarray(inputs["W_out"], np.float32).T        # [F, D]
    w_out = np.ascontiguousarray(
        WoT.reshape(MFO, 128, MD, 128).transpose(2, 1, 0, 3)).astype(np.float16)
    return {
        "w_mix": w_mix, "b_mix": b_mix, "w_gu": w_gu, "w_out": w_out,
        "ident": np.eye(128, dtype=np.float32),
    }


def run(inputs, trace=False, **kw):
    x = np.asarray(inputs["x"], np.float32)
    shared = _prep_weights(inputs)
    in_maps = [dict(shared, x=np.ascontiguousarray(x[b])) for b in range(B)]
    res = run_bass_kernel_spmd(_get_nc(), in_maps, list(range(B)), trace=trace, **kw)
    out = np.stack([np.asarray(res.results[b]["out"], np.float32) for b in range(B)])
    return out, res


def kernel(**inputs) -> np.ndarray:
    out, _ = run(inputs)
    return out



# revision 14
# speedup vs baseline: 1.5728x; 1.5728x over previous
"""MinGRU block kernel for Trainium2 (Bass/Tile), SPMD over 8 NeuronCores.

Problem: B=8, S=2048, D=1024, F=3072 (nn_MinGRUBlock).
Sharding: data-parallel over batch (one batch row per core); weights replicated.

v2: all six matmul groups run in fp8(e4m3) with DoubleRow perf mode (2 fp8
weights per PE cell -> 2x MAC throughput), N=512 free dims. Weight tensors are
pre-scaled by power-of-2 factors into fp8 range; the inverse scales fold into
the (already present) ACT readout scale constants, so descaling is free.
Activations are quantized to fp8 with an 8x scale folded into the rmsnorm
reciprocal; the FFN z tile carries a 16x scale compensated at the final
residual readout (the residual scratch is written 131072x scaled via the
tensor_tensor_reduce output scale so the phase-2b add stays scale-consistent).

Per-core dataflow (compute in "T layout": feature on partitions, time free):
  phase 1 (mixer, s-chunks of 512, per-chunk stages):
    A: load x chunk, PE-transpose to xT, ACT squares, PE ones-reduce (norm1)
    B: sqrt/recip, GPSIMD partition-broadcast, xnT = xT*r -> fp8
    C: g/d/v projections as fp8 DoubleRow matmuls (4 MMs of K=256 each),
       ACT sigmoid/tanh readouts (tables batched per proj pass), DVE
       tensor_tensor_scan with fp32 carry, out1s = (x+h)*2^17, norm2 squares
    D: norm2 sqrt/recip/broadcast, o1n = out1s*r -> fp8 resident
  phase 2a: z = silu(gate)*up*16 in fp8 (gate via fused ACT Silu, z via
    DVE tensor_tensor_reduce reading the up PSUM directly)
  phase 2b: W_out DoubleRow matmuls + residual add + PE-transpose back,
    final 1/2^17 descale on the DVE copy out of transpose PSUM.
"""

import os
import sys
from contextlib import ExitStack

import numpy as np
import ml_dtypes

for _p in ("/opt/trn_rl_repo", "/root/.axon_site/_ro/trn_rl_repo"):
    if os.path.isdir(_p) and _p not in sys.path:
        sys.path.insert(0, _p)

import concourse.bass as bass
import concourse.tile as tile
from concourse import bacc, mybir
from concourse.bass_utils import run_bass_kernel_spmd

F32 = mybir.dt.float32
F16 = mybir.dt.float16
F8 = mybir.dt.float8e4
AF = mybir.ActivationFunctionType
OP = mybir.AluOpType
DR = mybir.MatmulPerfMode.DoubleRow

B, S, D, F = 8, 2048, 1024, 3072
EPS = 1e-6
KD = D // 128           # 8 d-ptiles
MD = D // 128           # 8
MFO = F // 128          # 24 f-ptiles
MF2 = 2 * F // 128      # 48 (gate|up)

CH = 512                # s-chunk (both phases)
NCH = S // CH           # 4
NST = CH // 128         # 4 s-tiles per chunk

# fp8 scaling constants (fixed powers of two; inputs are bounded by
# construction: |W{g,v,d}|<=1/32, |W_gate/up|<=1/32, |W_out|<=1/sqrt(3072))
AS = 8.0                # activation quantization scale (normalized acts)
S_MIX = 4096.0          # mixer weight scale      -> |w|*S <= 128
S_GU = 4096.0           # gate weight scale       -> |w|*S <= 128
S_UP = 4.0              # up weight scale (small so z = gate*ups fits fp8)
S_O = 8192.0            # out-proj weight scale   -> |w|*S <= 148
ZETA = AS * S_UP        # scale carried by the fp8 z tile (= 32)


def build_program():
    nc = bacc.Bacc("TRN2", target_bir_lowering=False, debug=False)

    x_d = nc.dram_tensor("x", [S, D], F32, kind="ExternalInput").ap()
    wmix_d = nc.dram_tensor("w_mix", [3 * MD, 128, KD, 128], F8, kind="ExternalInput").ap()
    bmix_d = nc.dram_tensor("b_mix", [128, 3 * MD], F32, kind="ExternalInput").ap()
    wgu_d = nc.dram_tensor("w_gu", [MF2, 128, KD, 128], F8, kind="ExternalInput").ap()
    wout_d = nc.dram_tensor("w_out", [MD, 128, MFO, 128], F8, kind="ExternalInput").ap()
    ident_d = nc.dram_tensor("ident", [128, 128], F32, kind="ExternalInput").ap()
    out_d = nc.dram_tensor("out", [S, D], F32, kind="ExternalOutput").ap()

    with tile.TileContext(nc) as tc, ExitStack() as top:
        # ---------- persistent tiles ----------
        cpool = top.enter_context(tc.tile_pool(name="consts", bufs=1))
        ident = cpool.tile([128, 128], F32)
        nc.sync.dma_start(ident[:], ident_d[:])
        ones_col = cpool.tile([128, 1], F16)
        nc.vector.memset(ones_col[:], 1.0)
        bmix = cpool.tile([128, 3 * MD], F32)
        nc.sync.dma_start(bmix[:], bmix_d[:])
        eps1 = cpool.tile([1, 1], F32)
        nc.vector.memset(eps1[:], EPS / (AS * AS))

        # DRAM scratch for the (scaled) mixer output residual, per chunk
        dpool = top.enter_context(tc.tile_pool(name="dscratch", bufs=1, space="DRAM"))
        sc1_t = [dpool.tile([KD, 128, CH], F32, name=f"sc1_{i}") for i in range(NCH)]

        # normalized out1 (x8) stays resident in SBUF across phase 1 -> 2a
        o1n_pool = top.enter_context(tc.tile_pool(name="o1n", bufs=1))
        o1n = o1n_pool.tile([128, KD, S], F8)

        carry_pool = top.enter_context(tc.tile_pool(name="carry", bufs=1))
        carry = carry_pool.tile([128, KD], F32)

        # ---------- phase 1: mixer ----------
        with ExitStack() as ph1:
            wpool = ph1.enter_context(tc.tile_pool(name="wmix", bufs=1))
            wmix = wpool.tile([128, 3 * MD, KD, 128], F8)
            wmix_dp = wmix_d.rearrange("m p k j -> p m k j")

            p_nat = ph1.enter_context(tc.tile_pool(name="xnat", bufs=5))
            p_xT = ph1.enter_context(tc.tile_pool(name="xT", bufs=2))
            p_x8 = ph1.enter_context(tc.tile_pool(name="x8", bufs=2))
            p_sq = ph1.enter_context(tc.tile_pool(name="sq", bufs=2))
            p_sg = ph1.enter_context(tc.tile_pool(name="sg", bufs=2))
            p_at = ph1.enter_context(tc.tile_pool(name="at", bufs=2))
            p_sm = ph1.enter_context(tc.tile_pool(name="sm", bufs=2))
            p_hT = ph1.enter_context(tc.tile_pool(name="hT", bufs=2))
            p_o1 = ph1.enter_context(tc.tile_pool(name="o1", bufs=2))
            p_row = ph1.enter_context(tc.tile_pool(name="rows", bufs=2))
            p_bc = ph1.enter_context(tc.tile_pool(name="bc", bufs=2))
            ps_tp = ph1.enter_context(tc.tile_pool(name="tp_ps", bufs=2, space="PSUM"))
            ps_mm = ph1.enter_context(tc.tile_pool(name="mm_ps", bufs=3, space="PSUM"))
            ps_ss = ph1.enter_context(tc.tile_pool(name="ss_ps", bufs=2, space="PSUM"))

            st = {}  # c -> dict of live tiles

            def stA(c):
                """load + transpose x chunk; norm1 squares + PE reduce."""
                s0 = c * CH
                d = st.setdefault(c, {})
                xT = p_xT.tile([128, KD, CH], F32, tag="xT", name=f"xT{c}")
                nats = []
                for stt in range(NST):
                    nat = p_nat.tile([128, D], F32, tag="nat", name=f"nat{c}_{stt}")
                    nc.sync.dma_start(nat[:], x_d[s0 + stt * 128: s0 + (stt + 1) * 128, :])
                    nats.append(nat)
                ss1 = ps_ss.tile([1, CH], F32, tag="ss", name=f"ss1_{c}")
                for kt in range(KD):
                    tp = ps_tp.tile([128, CH], F32, tag="tp", name=f"tp{c}_{kt}")
                    for stt in range(NST):
                        nc.tensor.transpose(tp[:, stt * 128:(stt + 1) * 128],
                                            nats[stt][:, kt * 128:(kt + 1) * 128],
                                            ident[:])
                    nc.vector.tensor_copy(xT[:, kt], tp[:])
                    sq = p_sq.tile([128, CH], F16, tag="sq1", name=f"sq1_{c}_{kt}")
                    nc.scalar.activation(sq[:], xT[:, kt], AF.Square, bias=0.0)
                    nc.tensor.matmul(ss1[:], ones_col[:], sq[:],
                                     start=(kt == 0), stop=(kt == KD - 1))
                d["xT"] = xT
                d["ss1"] = ss1

            def stB(c):
                """norm1 scale; xnT = AS * x / rms -> fp8."""
                d = st[c]
                srow = p_row.tile([1, CH], F32, tag="srow1", name=f"srow1_{c}")
                nc.scalar.activation(srow[:], d["ss1"][:], AF.Sqrt,
                                     bias=eps1[:], scale=1.0 / (AS * AS * D))
                rrow = p_row.tile([1, CH], F32, tag="rrow1", name=f"rrow1_{c}")
                nc.vector.reciprocal(rrow[:], srow[:])
                bc = p_bc.tile([128, CH], F32, tag="bc1", name=f"bc1_{c}")
                nc.gpsimd.partition_broadcast(bc[:], rrow[:])
                xnT = p_x8.tile([128, KD, CH], F8, tag="xnT", name=f"xnT{c}")
                for kt in range(KD):
                    nc.vector.tensor_tensor(xnT[:, kt], d["xT"][:, kt], bc[:], OP.mult)
                d["xnT"] = xnT

            def _proj(d, mt, out_ap, fn):
                ps = ps_mm.tile([128, CH], F32, tag="mm", name=f"mm_{mt}")
                for j in range(KD // 2):
                    nc.tensor.matmul(ps[:], wmix[:, mt, 2 * j:2 * j + 2, :],
                                     d["xnT"][:, 2 * j:2 * j + 2, :],
                                     start=(j == 0), stop=(j == KD // 2 - 1),
                                     perf_mode=DR)
                nc.scalar.activation(out_ap, ps[:], fn,
                                     bias=bmix[:, mt:mt + 1], scale=1.0 / (AS * S_MIX))

            def stC(c):
                """projections (fp8 DoubleRow), activations, scan, residual."""
                d = st[c]
                sg = p_sg.tile([128, KD, CH], F16, tag="sg", name=f"sg{c}")
                for kt in range(KD):          # g-pass (sigmoid table)
                    _proj(d, kt, sg[:, kt], AF.Sigmoid)
                a_t = p_at.tile([128, KD, CH], F16, tag="at", name=f"at{c}")
                for kt in range(KD):          # d-pass (sigmoid table)
                    sd = p_sm.tile([128, CH], F16, tag="sd", name=f"sd{c}_{kt}")
                    _proj(d, 2 * MD + kt, sd[:], AF.Sigmoid)
                    nc.vector.tensor_scalar(a_t[:, kt], sd[:], 0.998, 0.001,
                                            OP.mult, OP.add)
                out1 = p_o1.tile([128, KD, CH], F32, tag="o1", name=f"o1_{c}")
                for kt in range(KD):          # v-pass (tanh table) + scan chain
                    tv = p_sm.tile([128, CH], F16, tag="tv", name=f"tv{c}_{kt}")
                    _proj(d, MD + kt, tv[:], AF.Tanh)
                    xs = p_sm.tile([128, CH], F16, tag="xs", name=f"xs{c}_{kt}")
                    nc.vector.tensor_tensor(xs[:], sg[:, kt], tv[:], OP.mult)
                    hT = p_hT.tile([128, CH], F32, tag="hT", name=f"hT{c}_{kt}")
                    init = 0.0 if c == 0 else carry[:, kt:kt + 1]
                    nc.vector.tensor_tensor_scan(hT[:], a_t[:, kt], xs[:],
                                                 init, OP.mult, OP.add)
                    nc.vector.tensor_copy(carry[:, kt:kt + 1], hT[:, CH - 1:CH])
                    nc.vector.tensor_tensor(out1[:, kt], d["xT"][:, kt], hT[:],
                                            OP.add)
                    nc.sync.dma_start(sc1_t[c][kt], out1[:, kt])
                ss2 = ps_ss.tile([1, CH], F32, tag="ss", name=f"ss2_{c}")
                for kt in range(KD):          # norm2 squares (square table)
                    sq = p_sq.tile([128, CH], F16, tag="sq2", name=f"sq2_{c}_{kt}")
                    nc.scalar.activation(sq[:], out1[:, kt], AF.Square, bias=0.0)
                    nc.tensor.matmul(ss2[:], ones_col[:], sq[:],
                                     start=(kt == 0), stop=(kt == KD - 1))
                d["out1"] = out1
                d["ss2"] = ss2

            def stD(c):
                """norm2 scale; o1n = AS * out1 / rms -> fp8 resident."""
                d = st[c]
                s0 = c * CH
                srow = p_row.tile([1, CH], F32, tag="srow2", name=f"srow2_{c}")
                nc.scalar.activation(srow[:], d["ss2"][:], AF.Sqrt,
                                     bias=eps1[:], scale=1.0 / (AS * AS * D))
                rrow = p_row.tile([1, CH], F32, tag="rrow2", name=f"rrow2_{c}")
                nc.vector.reciprocal(rrow[:], srow[:])
                bc = p_bc.tile([128, CH], F32, tag="bc2", name=f"bc2_{c}")
                nc.gpsimd.partition_broadcast(bc[:], rrow[:])
                for kt in range(KD):
                    nc.vector.tensor_tensor(o1n[:, kt, s0:s0 + CH],
                                            d["out1"][:, kt], bc[:], OP.mult)
                del st[c]

            stA(0)
            # mixer weights per e-ptile so the first projections start early
            for mt in range(3 * MD):
                nc.sync.dma_start(wmix[:, mt], wmix_dp[:, mt])
            stB(0)
            for c in range(NCH):
                if c + 1 < NCH:
                    stA(c + 1)
                stC(c)
                if c + 1 < NCH:
                    stB(c + 1)
                stD(c)

        # ---------- phase 2: FFN ----------
        with ExitStack() as ph2:
            zpool = ph2.enter_context(tc.tile_pool(name="zbuf", bufs=1))
            z = zpool.tile([128, MFO, S], F8)
            wopool = ph2.enter_context(tc.tile_pool(name="wout", bufs=1))
            wout = wopool.tile([128, MD, MFO, 128], F8)
            nc.sync.dma_start(wout[:], wout_d.rearrange("m p k j -> p m k j"))

            # 2a: gate/up + z
            with ExitStack() as ph2a:
                p_wgu = ph2a.enter_context(tc.tile_pool(name="wgu", bufs=4))
                p_gt = ph2a.enter_context(tc.tile_pool(name="gt", bufs=3))
                ps_gu = ph2a.enter_context(tc.tile_pool(name="gu_ps", bufs=4, space="PSUM"))
                for mg in range(MFO):
                    wg = p_wgu.tile([128, KD, 128], F8, tag="wgu")
                    nc.sync.dma_start(wg[:], wgu_d[mg])
                    wu = p_wgu.tile([128, KD, 128], F8, tag="wgu")
                    nc.sync.dma_start(wu[:], wgu_d[MFO + mg])
                    for sc in range(NCH):
                        sl = slice(sc * CH, (sc + 1) * CH)
                        gps = ps_gu.tile([128, CH], F32, tag="gups")
                        for j in range(KD // 2):
                            nc.tensor.matmul(gps[:], wg[:, 2 * j:2 * j + 2, :],
                                             o1n[:, 2 * j:2 * j + 2, sl],
                                             start=(j == 0), stop=(j == KD // 2 - 1),
                                             perf_mode=DR)
                        ups = ps_gu.tile([128, CH], F32, tag="gups")
                        for j in range(KD // 2):
                            nc.tensor.matmul(ups[:], wu[:, 2 * j:2 * j + 2, :],
                                             o1n[:, 2 * j:2 * j + 2, sl],
                                             start=(j == 0), stop=(j == KD // 2 - 1),
                                             perf_mode=DR)
                        gate = p_gt.tile([128, CH], F16, tag="gate")
                        nc.scalar.activation(gate[:], gps[:], AF.Silu,
                                             bias=0.0, scale=1.0 / (AS * S_GU))
                        # z = silu(G) * (AS*S_UP*U): fp8 tile carries ZETA=32
                        nc.vector.tensor_tensor(z[:, mg, sl], gate[:], ups[:],
                                                OP.mult)

            # 2b: W_out + residual + transpose out
            with ExitStack() as ph2b:
                p_o1c = ph2b.enter_context(tc.tile_pool(name="o1c", bufs=3))
                p_yy = ph2b.enter_context(tc.tile_pool(name="yy", bufs=2))
                p_oT = ph2b.enter_context(tc.tile_pool(name="outT", bufs=MD + 1))
                p_onat = ph2b.enter_context(tc.tile_pool(name="onat", bufs=3))
                ps_y = ph2b.enter_context(tc.tile_pool(name="y_ps", bufs=2, space="PSUM"))
                ps_t2 = ph2b.enter_context(tc.tile_pool(name="t2_ps", bufs=2, space="PSUM"))
                for sc in range(NCH):
                    sl = slice(sc * CH, (sc + 1) * CH)
                    outTs = []
                    for mo in range(MD):
                        yps = ps_y.tile([128, CH], F32, tag="yps")
                        for j in range(MFO // 2):
                            nc.tensor.matmul(yps[:], wout[:, mo, 2 * j:2 * j + 2, :],
                                             z[:, 2 * j:2 * j + 2, sl],
                                             start=(j == 0), stop=(j == MFO // 2 - 1),
                                             perf_mode=DR)
                        o1c = p_o1c.tile([128, CH], F32, tag="o1c")
                        nc.sync.dma_start(o1c[:], sc1_t[sc][mo])
                        yy = p_yy.tile([128, CH], F32, tag="yy")
                        nc.vector.tensor_scalar(yy[:], yps[:], 1.0 / (ZETA * S_O),
                                                0.0, OP.mult, OP.add)
                        oT = p_oT.tile([128, CH], F32, tag="oT")
                        nc.vector.tensor_tensor(oT[:], yy[:], o1c[:], OP.add)
                        outTs.append(oT)
                    for q in range(NST):
                        onat = p_onat.tile([128, D], F32, tag="onat")
                        for h in range(2):
                            t2 = ps_t2.tile([128, 512], F32, tag="t2")
                            for jj in range(4):
                                nc.tensor.transpose(
                                    t2[:, jj * 128:(jj + 1) * 128],
                                    outTs[4 * h + jj][:, q * 128:(q + 1) * 128],
                                    ident[:])
                            nc.vector.tensor_copy(onat[:, h * 512:(h + 1) * 512],
                                                  t2[:])
                        srow0 = sc * CH + q * 128
                        nc.sync.dma_start(out_d[srow0:srow0 + 128, :], onat[:])

    nc.compile()
    return nc


_NC = None


def _get_nc():
    global _NC
    if _NC is None:
        _NC = build_program()
    return _NC


def _q8(a, s):
    return np.clip(np.asarray(a, np.float32) * s, -240.0, 240.0).astype(
        ml_dtypes.float8_e4m3)


def _prep_weights(inputs):
    w1 = np.asarray(inputs["rms_mix_w"], np.float32)
    w2 = np.asarray(inputs["rms_ffn_w"], np.float32)
    Wg = np.asarray(inputs["Wg"], np.float32) * w1[None, :]
    Wv = np.asarray(inputs["Wv"], np.float32) * w1[None, :]
    Wd = np.asarray(inputs["Wd"], np.float32) * w1[None, :]
    Wcat = np.concatenate([Wg, Wv, Wd], axis=0)            # [3D, D]
    w_mix = _q8(np.ascontiguousarray(
        Wcat.T.reshape(KD, 128, 3 * MD, 128).transpose(2, 1, 0, 3)), S_MIX)
    bcat = np.concatenate([np.asarray(inputs["bg"], np.float32),
                           np.asarray(inputs["bv"], np.float32),
                           np.asarray(inputs["bd"], np.float32)])
    b_mix = np.ascontiguousarray(bcat.reshape(3 * MD, 128).T).astype(np.float32)
    Wgate = np.asarray(inputs["W_gate"], np.float32) * w2[None, :]
    Wup = np.asarray(inputs["W_up"], np.float32) * w2[None, :]
    Wcat2 = np.concatenate([Wgate * S_GU, Wup * S_UP], axis=0)  # [2F, D], pre-scaled
    w_gu = _q8(np.ascontiguousarray(
        Wcat2.T.reshape(KD, 128, MF2, 128).transpose(2, 1, 0, 3)), 1.0)
    WoT = np.asarray(inputs["W_out"], np.float32).T        # [F, D]
    w_out = _q8(np.ascontiguousarray(
        WoT.reshape(MFO, 128, MD, 128).transpose(2, 1, 0, 3)), S_O)
    return {
        "w_mix": w_mix, "b_mix": b_mix, "w_gu": w_gu, "w_out": w_out,
        "ident": np.eye(128, dtype=np.float32),
    }


def run(inputs, trace=False, **kw):
    x = np.asarray(inputs["x"], np.float32)
    shared = _prep_weights(inputs)
    in_maps = [dict(shared, x=np.ascontiguousarray(x[b])) for b in range(B)]
    res = run_bass_kernel_spmd(_get_nc(), in_maps, list(range(B)), trace=trace, **kw)
    out = np.stack([np.asarray(res.results[b]["out"], np.float32) for b in range(B)])
    return out, res


def kernel(**inputs) -> np.ndarray:
    out, _ = run(inputs)
    return out
